# revision 42
# baseline (speedup 1.0000x reference)
"""Trainium2 Bass kernel for 16-head MHA (b=2, n=2048, c=1024, d=64).

Reference semantics (note the inverted scale "bug" reproduced faithfully):
    qkv = x @ W_qkv + b_qkv
    scores = (q @ k^T) * sqrt(d)          # multiplied, not divided
    out = softmax(scores) @ v
    y = concat_heads(out) @ W_proj + b_proj

Sharding: tensor-parallel over heads. Each of the 8 cores computes QKV +
attention for its 2 heads (head-dim-transposed layouts so no activation
transposes are needed beyond one x^T pass), then a per-batch AllToAll moves
the per-head attention outputs into a row-sharded layout and each core
computes the final projection for its 512-row output shard. Host
concatenates shards.

Precision strategy (validated against the 2e-2 gate by numpy simulation of
the full rounding chain): QKV runs 2 matmul passes with an fp16 hi/lo split
of the *weights* (split once at load; x is a single fp16 cast), and the
transposed score matmul runs a single fp16 pass (SC2 flips it to the
baseline hi*hi+cross 2-pass form if more margin is ever needed). Simulated
absmax rel err: 8.6e-3 one-pass / 6.2e-3 two-pass vs the 2e-2 gate. The
row-max pass needs no precision at all (softmax renormalization cancels any
max offset exactly). exp scores, A@V and the projection are plain fp16.

Softmax plumbing: q^T and k^T live in 65-partition tiles [65, b*h, n]; row
64 of k^T is ones and row 64 of q^T holds -rowmax, so the K=65 transposed
score matmul lands exp-ready in PSUM. V gets an extra ones *column* so the
A@V matmul also yields the softmax denominators; a DVE fast-reciprocal
(partition-0 copy first; no ACT table loads) + PE partition-broadcast +
multiply normalizes each head-output block.

Pipelining: stage 0's row-max pass runs inside QKV chunks 4..6 (chunk 7
stays dense so the PE clock survives the phase boundary, and the stats
flatten DMA lands under it); stage i+1's max pass interleaves stage i's
transposed pass as half-groups (2 matmuls + one batched 2-bank DVE reduce)
every other k-tile, stall-free at 2 psum bufs; A@V runs one k-tile behind
the score matmuls so the PE never waits on ACT's exp; each stage's output
quarter is staged for its AllToAll right after its norm, and batch-0's
output projection runs as stage-3 fillers so only batch-1's projection
waits on the final collective.
"""

import sys
from contextlib import ExitStack

sys.path.insert(0, "/opt/trn_rl_repo")

import numpy as np

import concourse.bass as bass
import concourse.tile as tile
from concourse import bacc, mybir
from concourse import bass_utils
from concourse.masks import make_identity

# Problem shape (hardcoded per contract)
B, N, C = 2, 2048, 1024
H, D = 16, 64
NCORES = 8
HPC = H // NCORES          # heads per core = 2
D2 = HPC * D               # 128 = per-core slice of the concat dim
R = B * N                  # 4096 flattened rows
RS = R // NCORES           # 512 output rows per core
KS = C // 128              # 8 contraction blocks of 128
CHUNK = 512                # rows per x^T/qkv chunk
NCH = R // CHUNK           # 8 chunks
NQT = N // 128             # 16 query tiles per batch
NKT = N // 128             # 16 key tiles per batch
F32 = mybir.dt.float32
F16 = mybir.dt.float16

INV_SCALE = float(np.sqrt(D))  # 8.0, multiplied into q
SC2 = False                # True: 2-pass (hi*hi + cross) transposed scores


def _bcast(ap, parts):
    """Broadcast a DRAM AP across `parts` partitions (step-0 partition dim)."""
    return bass.AP(tensor=ap.tensor, offset=ap.offset,
                   ap=[[0, parts]] + list(ap.ap))


def build_program():
    nc = bacc.Bacc("TRN2", target_bir_lowering=False, debug=False,
                   num_devices=NCORES)

    xT_in = nc.dram_tensor("xT", [C, R], F32, kind="ExternalInput")
    wq_in = nc.dram_tensor("wq", [C, D2], F32, kind="ExternalInput")
    wk_in = nc.dram_tensor("wk", [C, D2], F32, kind="ExternalInput")
    wv_in = nc.dram_tensor("wv", [C, D2], F32, kind="ExternalInput")
    bq_in = nc.dram_tensor("bq", [D2], F32, kind="ExternalInput")
    bk_in = nc.dram_tensor("bk", [D2], F32, kind="ExternalInput")
    bv_in = nc.dram_tensor("bv", [D2], F32, kind="ExternalInput")
    wp_in = nc.dram_tensor("wp", [C, C], F32, kind="ExternalInput")
    bp_in = nc.dram_tensor("bp", [C], F32, kind="ExternalInput")
    out_t = nc.dram_tensor("out", [RS, C], F32, kind="ExternalOutput")

    with tile.TileContext(nc) as tc:
        kernel_body(tc, xT_in, wq_in, wk_in, wv_in, bq_in, bk_in, bv_in,
                    wp_in, bp_in, out_t)
    nc.compile()
    return nc


def kernel_body(tc, xT_in, wq_in, wk_in, wv_in, bq_in, bk_in, bv_in,
                wp_in, bp_in, out_t):
    nc = tc.nc
    Exp = mybir.ActivationFunctionType.Exp
    Ident = mybir.ActivationFunctionType.Identity

    ctx = ExitStack()
    consts = ctx.enter_context(tc.tile_pool(name="consts", bufs=1))
    persist = ctx.enter_context(tc.tile_pool(name="persist", bufs=1))
    dram = ctx.enter_context(tc.tile_pool(name="dram", bufs=1, space="DRAM"))

    ident = consts.tile([128, 128], F32)
    make_identity(nc, ident)
    ones64 = consts.tile([1, 64], F16)
    nc.vector.memset(ones64, 1.0)

    # --- weights / biases for qkv (hi/lo split of w in fp16; x single) ---
    with tc.tile_pool(name="wstage", bufs=2) as wstage:
        def split_w(name, t_in):
            w_f32 = wstage.tile([128, KS, D2], F32, tag="w_f32", name=name)
            nc.gpsimd.dma_start(w_f32,
                               t_in.ap().rearrange("(ks p) m -> p ks m", p=128))
            hi = consts.tile([128, KS, D2], F16, name=name + "_hi")
            lo = consts.tile([128, KS, D2], F16, name=name + "_lo")
            nc.vector.tensor_copy(hi, w_f32)
            nc.vector.tensor_sub(lo, w_f32, hi)
            return hi, lo

        wq_hi, wq_lo = split_w("wq", wq_in)
        wk_hi, wk_lo = split_w("wk", wk_in)
        wv_f32 = wstage.tile([128, KS, D2], F32, tag="w_f32", name="wv")
        nc.gpsimd.dma_start(wv_f32, wv_in.ap().rearrange("(ks p) m -> p ks m", p=128))
        wv_bf = consts.tile([128, KS, D2], F16)
        nc.vector.tensor_copy(wv_bf, wv_f32)

    # per-head bias columns on partitions 0..63: [64, HPC]
    bq2 = consts.tile([64, HPC], F32)
    bk2 = consts.tile([64, HPC], F32)
    nc.gpsimd.dma_start(bq2, bq_in.ap().rearrange("(h p) -> p h", p=64))
    nc.gpsimd.dma_start(bk2, bk_in.ap().rearrange("(h p) -> p h", p=64))
    bq2_8 = consts.tile([64, HPC], F32)
    nc.scalar.mul(bq2_8, bq2, INV_SCALE)
    bv_sb = consts.tile([128, D2], F32)
    nc.gpsimd.dma_start(bv_sb, _bcast(bv_in.ap(), 128))

    # --- persistent activations (fp16) ---
    # qTx/kTx: [65, b*HPC+h, n] — rows 0..63 = (scaled) q^T / k^T for that
    # (batch, head); kTx row 64 = ones, qTx row 64 = -rowmax (filled per
    # stage by the stats flatten).
    qTx = persist.tile([65, B * HPC, N], F16)
    kTx = persist.tile([65, B * HPC, N], F16)
    nc.vector.memset(kTx[64:65], 1.0)
    if SC2:
        qTl = persist.tile([64, B * HPC, N], F16)   # q^T lo residual
        kTl = persist.tile([64, B * HPC, N], F16)   # k^T lo residual
    # v with a ones column per head: [p, row_tile, head, 65]
    v_sb = persist.tile([128, R // 128, HPC, D + 1], F16)
    nc.vector.memset(v_sb[:, :, :, D:D + 1], 1.0)
    outT_sb = persist.tile([128, R], F16)

    SEQ = [(b, h) for b in range(B) for h in range(HPC)]
    attctx = ExitStack()
    attE = ctx.enter_context(tc.tile_pool(name="attE", bufs=1))

    stats_t = {}   # stage -> stats tile [128, 16] (negated row maxes)
    av_t = {}      # stage -> [65, 4, 512] unnormalized head outputs + sums

    mx_t = {}

    def emit_max_half(i, mt, half, pool, ps_bufs):
        # half a row-max group: 2 matmuls into a 2-bank psum tile, ONE
        # batched DVE reduce (911ns vs 2x643ns single-bank) into mx; the
        # second half also folds mx into the stats column. Halves are
        # emitted at separate slots in the main PE stream so the psum-free
        # -> reduce roundtrip always hides behind main-stream work.
        b, h = SEQ[i]
        bh = b * HPC + h
        if mt == 0 and half == 0:
            stats_t[i] = attE.tile([128, 16], F32, tag="stats", bufs=2,
                                  name=f"stats{i}")
        if half == 0:
            mx_t[i] = attE.tile([128, 2], F32, tag="mx", bufs=2, name="mx")
        q_l = qTx[0:64, bh, mt * 128:(mt + 1) * 128]
        p = pool.tile([128, 2, 512], F32, tag="ps2", bufs=ps_bufs, name="ps2")
        for jj in range(2):
            j = half * 2 + jj
            nc.tensor.matmul(
                p[:, jj], q_l, kTx[0:64, bh, j * 512:(j + 1) * 512],
                start=True, stop=True)
        nc.vector.reduce_max(mx_t[i][:, half:half + 1], p,
                             axis=mybir.AxisListType.XY)
        if half == 1:
            nc.vector.reduce_max(stats_t[i][:, mt:mt + 1], mx_t[i],
                                 axis=mybir.AxisListType.X, negate=True)

    def emit_stats_flatten(i, pool, tag, tag_bufs=1):
        b, h = SEQ[i]
        bh = b * HPC + h
        stats = stats_t.pop(i)
        pstat = pool.tile([16, 128], F32, tag=tag, bufs=tag_bufs,
                          name="pstat")
        nc.tensor.transpose(pstat, stats, ident)
        statsT = attE.tile([16, 128], F16, tag="statsT", bufs=2, name="statsT")
        nc.vector.tensor_copy(statsT, pstat)
        nc.sync.dma_start(
            qTx[64:65, bh, :].rearrange("s (m q) -> s m q", m=16), statsT)

    # ---------- Phase 1+2: x^T chunks and QKV projections ----------
    # stage-0 max-pass halves are spread over chunks 4..6 (chunk 7 stays a
    # dense pure-QKV PE stream and the stage-0 stats flatten DMA lands
    # during it): any >3.4us PE gap at the phase boundary re-throttles the
    # PE clock to 1.2 GHz and the attention stages then never re-warm.
    s0_fill = [(mt, hf) for mt in range(NQT) for hf in range(2)]
    s0_counts = {4: [3, 3, 2, 2], 5: [3, 3, 2, 2], 6: [3, 3, 3, 3]}
    xT_view = xT_in.ap().rearrange("(ks p) r -> p ks r", p=128)
    with tc.tile_pool(name="xload", bufs=2) as xload, \
         tc.tile_pool(name="p12", bufs=1, space="PSUM") as p12:
        for ch in range(NCH):
            r0 = ch * CHUNK
            b = ch // (NCH // B)
            n0 = r0 - b * N      # column offset within the batch
            xT = xload.tile([128, KS, CHUNK], F32, tag="xT")
            x16 = xload.tile([128, KS, CHUNK], F16, tag="x16")
            # load + cast in pieces so compute starts before the chunk lands
            npc = 8 if ch == 0 else 2
            for hf in range(npc):
                ksl = slice(hf * KS // npc, (hf + 1) * KS // npc)
                nc.sync.dma_start(xT[:, ksl], xT_view[:, ksl, r0:r0 + CHUNK])
                nc.scalar.copy(x16[:, ksl], xT[:, ksl])
            # q^T, k^T for this chunk: 2 passes (w hi/lo), head-split on ACT
            for (whi, wlo, dstx, bias2, scale) in (
                    (wq_hi, wq_lo, qTx, bq2_8, INV_SCALE),
                    (wk_hi, wk_lo, kTx, bk2, 1.0)):
                pqk = p12.tile([128, CHUNK], F32, tag="pqk", bufs=2)
                for pi, w_p in enumerate((whi, wlo)):
                    for ks in range(KS):
                        nc.tensor.matmul(pqk, w_p[:, ks], x16[:, ks],
                                         start=(pi == 0 and ks == 0),
                                         stop=(pi == 1 and ks == KS - 1))
                for h in range(HPC):
                    nc.scalar.activation(
                        dstx[0:64, b * HPC + h, n0:n0 + CHUNK],
                        pqk[h * D:(h + 1) * D], Ident,
                        bias=bias2[:, h:h + 1], scale=scale)
                if SC2:
                    dstl = qTl if dstx is qTx else kTl
                    tmp = xload.tile([128, CHUNK], F32, tag="qk_tmp", bufs=2)
                    nc.vector.tensor_scalar(
                        tmp[0:64], pqk[0:64], scalar1=scale,
                        scalar2=bias2[:, 0:1], op0=mybir.AluOpType.mult,
                        op1=mybir.AluOpType.add)
                    nc.vector.tensor_scalar(
                        tmp[64:128], pqk[64:128], scalar1=scale,
                        scalar2=bias2[:, 1:2], op0=mybir.AluOpType.mult,
                        op1=mybir.AluOpType.add)
                    for h in range(HPC):
                        nc.gpsimd.tensor_sub(
                            dstl[:, b * HPC + h, n0:n0 + CHUNK],
                            tmp[h * D:(h + 1) * D],
                            dstx[0:64, b * HPC + h, n0:n0 + CHUNK])
            # v natural layout (single-pass fp16)
            for m in range(4):
                pv = p12.tile([128, D2], F32, tag="pv", bufs=1)
                for ks in range(KS):
                    nc.tensor.matmul(pv, x16[:, ks, m * 128:(m + 1) * 128],
                                     wv_bf[:, ks],
                                     start=(ks == 0), stop=(ks == KS - 1))
                for h in range(HPC):
                    nc.vector.tensor_add(
                        v_sb[:, ch * 4 + m, h, 0:D],
                        pv[:, h * D:(h + 1) * D],
                        bv_sb[:, h * D:(h + 1) * D])
                del pv
                # batch-0 q/k ready after chunk 3: overlap stage-0 max pass
                for _ in range(s0_counts.get(ch, [0] * 4)[m]):
                    mt, hf = s0_fill.pop(0)
                    emit_max_half(0, mt, hf, p12, 2)
            if ch == 6:
                emit_stats_flatten(0, p12, "pqk", tag_bufs=2)

    # projection weights: load + cast on the otherwise-idle gpsimd queue so
    # neither the sync DMA queue (stats flatten) nor the DVE queue (attention
    # reduces) stalls behind 4.5 MB of W_proj at the phase boundary
    projp = ctx.enter_context(tc.tile_pool(name="projp", bufs=1))
    wp_bf = projp.tile([128, KS, C], F16)
    with tc.tile_pool(name="wpl", bufs=2) as wpl:
        for ks in range(KS):
            wp_chunk = wpl.tile([128, C], F32, tag="wp_chunk")
            nc.gpsimd.dma_start(
                wp_chunk, wp_in.ap()[ks * 128:(ks + 1) * 128, :])
            nc.gpsimd.tensor_copy(wp_bf[:, ks], wp_chunk)
    bp_sb = projp.tile([128, C], F32)
    nc.gpsimd.dma_start(bp_sb, _bcast(bp_in.ap(), 128))

    # ---------- Phase 3: attention, software-pipelined over 4 stages ----------

    def emit_T_j(i, j, fillers):
        b, h = SEQ[i]
        bh = b * HPC + h
        if j == 0:
            av_t[i] = att.tile([65, 4, 512], F32, tag="av", bufs=2,
                               name=f"av{i}")
        qs = slice(j * 512, (j + 1) * 512)
        q_ext = qTx[:, bh, qs]
        if SC2:
            qcross = att.tile([128, 512], F16, tag="qcross", bufs=2,
                              name="qcross")
            nc.vector.tensor_copy(qcross[0:64], qTl[:, bh, qs])
            nc.vector.tensor_copy(qcross[64:128], qTx[0:64, bh, qs])
        pav = p3.tile([65, 512], F32, tag="pav", bufs=1, name="pav")
        # A@V for tile kt is emitted after the score matmul for kt+1 (one
        # k-tile of skew) so the PE never waits on ACT's exp of tile kt.
        pend = None
        for kt in range(NKT):
            psT = p3.tile([128, 512], F32, tag="psT", bufs=3, name="psT")
            kslc = slice(kt * 128, (kt + 1) * 128)
            if SC2:
                kcross = kcross_t[i]
                nc.tensor.matmul(psT, kTx[:, bh, kslc], q_ext,
                                 start=True, stop=False)
                nc.tensor.matmul(psT, kcross[:, kslc], qcross,
                                 start=False, stop=True)
            else:
                nc.tensor.matmul(psT, kTx[:, bh, kslc], q_ext,
                                 start=True, stop=True)
            eT = eTp.tile([128, 512], F16, tag="eT", bufs=4, name="eT")
            nc.scalar.activation(eT, psT, Exp)
            if pend is not None:
                pk, pe = pend
                nc.tensor.matmul(pav, v_sb[:, b * 16 + pk, h, :], pe,
                                 start=(pk == 0), stop=False)
            pend = (kt, eT)
            # one max-pass half-group every other k-tile: spreads the DVE
            # reduce load evenly through the stage, stall-free at bufs=2
            if kt % 2 == 1 and fillers:
                fillers.pop(0)()
        pk, pe = pend
        nc.tensor.matmul(pav, v_sb[:, b * 16 + pk, h, :], pe,
                         start=False, stop=(pk == NKT - 1))
        nc.vector.tensor_copy(av_t[i][:, j], pav)

    kcross_t = {}

    def emit_kcross(i):
        b, h = SEQ[i]
        bh = b * HPC + h
        kcross = att.tile([128, N], F16, tag="kcross", bufs=2, name="kcross")
        nc.vector.tensor_copy(kcross[0:64], kTx[0:64, bh, :])
        nc.vector.tensor_copy(kcross[64:128], kTl[:, bh, :])
        kcross_t[i] = kcross

    def emit_norm(i):
        b, h = SEQ[i]
        hp = h * D
        c0 = b * N
        av = av_t.pop(i)
        # 1/s: copy the denominators to partition 0 then the DVE
        # fast-reciprocal custom op (~18 correct bits, no ACT table loads
        # which cost ~1.3us per switch), broadcast across 64 partitions via
        # a K=1 outer product on the PE. rb rotates psum banks so rb[j+1]
        # overlaps the mul of j.
        rj0 = att.tile([1, 4, 512], F32, tag="rj0", bufs=2, name="rj0")
        nc.vector.tensor_copy(rj0, av[64:65, :, :])
        rjr = att.tile([1, 4, 512], F32, tag="rjr", bufs=2, name="rjr")
        nc.vector.reciprocal_approx_fast(rjr, rj0)
        rj16 = att.tile([1, 4, 512], F16, tag="rj16", bufs=2, name="rj16")
        nc.vector.tensor_copy(rj16, rjr)
        for j in range(4):
            qs = slice(c0 + j * 512, c0 + (j + 1) * 512)
            # broadcast 1/s across 64 partitions on the idle gpsimd (custom
            # partition-broadcast instr) instead of a K=1 PE outer product:
            # frees ~0.6us of PE per block and the stage-end psum banks
            rb = att.tile([64, 512], F16, tag="rb", bufs=2, name="rb")
            nc.gpsimd.partition_broadcast(rb, rj16[0:1, j])
            nc.vector.tensor_mul(outT_sb[hp:hp + D, qs], av[0:64, j], rb)

    # pipeline: stage 0's max pass ran inside the QKV phase; stage i+1's max
    # interleaves stage i's transposed pass (one pass per stage keeps the
    # DVE reduce load even), with its stats flattened at stage i+1's top
    att = attctx.enter_context(tc.tile_pool(name="att", bufs=1))
    eTp = attctx.enter_context(tc.tile_pool(name="eTp", bufs=1))
    p3 = attctx.enter_context(tc.tile_pool(name="p3", bufs=1, space="PSUM"))
    if SC2:
        emit_kcross(0)
    # per-batch AllToAll: batch b's shard j = its rows [j*256,(j+1)*256)
    # (core j's output = batch0 rows j*256.. plus batch1 rows j*256..)
    HRS = RS // 2  # 256 rows per batch per core
    a2a_in = [dram.tile([NCORES * 128, HRS], F16, name=f"a2ai{b}")
              for b in range(B)]
    a2a_out = [dram.tile([NCORES * 128, HRS], F16, name=f"a2ao{b}")
               for b in range(B)]
    lhsT_proj = [projp.tile([128, KS, HRS], F16, name=f"lhsTp{b}")
                 for b in range(B)]

    def emit_a2a_stage(b, hl):
        # stage this stage's output quarter into the collective buffer now
        nc.sync.dma_start(
            a2a_in[b].rearrange("(j p) r -> p j r",
                                j=NCORES)[hl * 64:(hl + 1) * 64],
            outT_sb[hl * 64:(hl + 1) * 64,
                    b * N:(b + 1) * N].rearrange("p (j r) -> p j r",
                                                 j=NCORES))

    def emit_a2a_coll(b):
        nc.gpsimd.collective_compute(
            "AllToAll", mybir.AluOpType.bypass,
            replica_groups=[list(range(NCORES))],
            ins=[a2a_in[b][:]], outs=[a2a_out[b][:]])
        for mh in range(2):
            msl = slice(mh * HRS // 2, (mh + 1) * HRS // 2)
            nc.sync.dma_start(
                lhsT_proj[b][:, :, msl],
                a2a_out[b].rearrange("(j p) r -> p j r", j=NCORES)[:, :, msl])

    projsb = attctx.enter_context(tc.tile_pool(name="projsb", bufs=1))

    def emit_proj_unit(m, nt):
        # one output-projection tile; batch-0 units run as stage-3 fillers
        # so the PE stays hot into the tail and only batch-1 waits on the
        # final collective
        lhsT_b = lhsT_proj[m // 2]
        mo = (m % 2) * 128
        pp = p3.tile([128, 512], F32, tag="ps2", bufs=2, name="pp")
        for ks in range(KS):
            nc.tensor.matmul(pp, lhsT_b[:, ks, mo:mo + 128],
                             wp_bf[:, ks, nt * 512:(nt + 1) * 512],
                             start=(ks == 0), stop=(ks == KS - 1))
        o_sb = projsb.tile([128, 512], F32, tag="o_sb", bufs=2)
        nc.vector.tensor_add(o_sb, pp, bp_sb[:, nt * 512:(nt + 1) * 512])
        nc.sync.dma_start(
            out_t.ap()[m * 128:(m + 1) * 128, nt * 512:(nt + 1) * 512],
            o_sb)

    fill_by_stage = {
        0: [(lambda mt=mt, hf=hf: emit_max_half(1, mt, hf, p3, 2))
            for mt in range(NQT) for hf in range(2)],
        1: [(lambda mt=mt, hf=hf: emit_max_half(2, mt, hf, p3, 2))
            for mt in range(NQT) for hf in range(2)],
        2: [(lambda mt=mt, hf=hf: emit_max_half(3, mt, hf, p3, 2))
            for mt in range(NQT) for hf in range(2)],
        3: [(lambda m=m, nt=nt: emit_proj_unit(m, nt))
            for m in range(2) for nt in range(2)],
    }

    def emit_warmers(n):
        # dense throwaway matmuls across the sparse stage boundary: keeps
        # the PE's HAM activity window busy so the 2.4 GHz clock survives
        # into the next stage (a cold stage runs its whole matmul stream
        # at 1.2 GHz, ~45us/stage)
        for w in range(n):
            wp_ = p3.tile([128, 512], F32, tag="ps2", bufs=2, name="warm")
            nc.tensor.matmul(wp_, kTx[0:64, 0, 0:128], kTx[0:64, 0, 0:512],
                             start=True, stop=True)

    for i in range(4):
        fillers = fill_by_stage[i]
        # stage i-1's fillers completed stage i's max pass; flatten now
        if i >= 1:
            emit_stats_flatten(i, p3, "pav")
            if SC2:
                emit_kcross(i)
        for j in range(4):
            emit_T_j(i, j, fillers)
        emit_norm(i)
        b_, h_ = SEQ[i]
        emit_a2a_stage(b_, h_)
        if i == 1:
            emit_a2a_coll(0)
        elif i == 3:
            emit_a2a_coll(1)
        if i < 3:
            emit_warmers(8)
    attctx.close()

    # ---------- Phase 4: batch-1 output projection (after A2A #2) ----------
    with tc.tile_pool(name="proj", bufs=1) as proj, \
         tc.tile_pool(name="p4", bufs=1, space="PSUM") as p4:
        for m in range(2, RS // 128):
            lhsT_b = lhsT_proj[m // 2]
            mo = (m % 2) * 128
            for nt in range(C // 512):
                pp = p4.tile([128, 512], F32, tag="pp", bufs=4)
                for ks in range(KS):
                    nc.tensor.matmul(pp, lhsT_b[:, ks, mo:mo + 128],
                                     wp_bf[:, ks, nt * 512:(nt + 1) * 512],
                                     start=(ks == 0), stop=(ks == KS - 1))
                o_sb = proj.tile([128, 512], F32, tag="o_sb", bufs=4)
                nc.vector.tensor_add(o_sb, pp,
                                     bp_sb[:, nt * 512:(nt + 1) * 512])
                nc.sync.dma_start(
                    out_t.ap()[m * 128:(m + 1) * 128, nt * 512:(nt + 1) * 512],
                    o_sb)
    ctx.close()


_PROGRAM = None


def _get_program():
    global _PROGRAM
    if _PROGRAM is None:
        _PROGRAM = build_program()
    return _PROGRAM


def kernel(x, W_qkv, b_qkv, W_proj, b_proj, _trace=False):
    xT = np.ascontiguousarray(np.asarray(x, dtype=np.float32).reshape(R, C).T)
    W_qkv = np.asarray(W_qkv, dtype=np.float32)
    b_qkv = np.asarray(b_qkv, dtype=np.float32)
    W_proj = np.ascontiguousarray(np.asarray(W_proj, dtype=np.float32))
    b_proj = np.ascontiguousarray(np.asarray(b_proj, dtype=np.float32))

    in_maps = []
    for i in range(NCORES):
        lo = i * D2            # first column of my heads within a qkv block
        hi = lo + D2
        in_maps.append({
            "xT": xT,
            "wq": np.ascontiguousarray(W_qkv[:, 0 * C + lo:0 * C + hi]),
            "wk": np.ascontiguousarray(W_qkv[:, 1 * C + lo:1 * C + hi]),
            "wv": np.ascontiguousarray(W_qkv[:, 2 * C + lo:2 * C + hi]),
            "bq": np.ascontiguousarray(b_qkv[0 * C + lo:0 * C + hi]),
            "bk": np.ascontiguousarray(b_qkv[1 * C + lo:1 * C + hi]),
            "bv": np.ascontiguousarray(b_qkv[2 * C + lo:2 * C + hi]),
            "wp": W_proj,
            "bp": b_proj,
        })

    nc = _get_program()
    res = bass_utils.run_bass_kernel_spmd(
        nc, in_maps, core_ids=list(range(NCORES)), trace=_trace)
    out = np.empty((R, C), dtype=np.float32)
    HRS = RS // 2
    for i in range(NCORES):
        o = res.results[i]["out"]
        for b in range(B):
            out[b * N + i * HRS: b * N + (i + 1) * HRS] = \
                o[b * HRS:(b + 1) * HRS]
    if _trace:
        kernel.last_results = res
    return out.reshape(B, N, C)
